# revision 20
# baseline (speedup 1.0000x reference)
"""Trainium2 Bass kernel for conditional-adjustment conv (CAConv).

Per sample b: h = relu(c[b] @ mlp_w1 + mlp_b1); adj = h @ mlp_w2 + mlp_b2;
w[b] = conv_w + adj.reshape(Co,Ci,3,3); out[b] = conv2d(x[b], w[b], pad=1) + conv_b.

Sharding: data-parallel over batch, 4 samples per core on 8 cores (SPMD).

Per-core device kernel:
  Stage A (weight gen, bf16): one small matmul + relu produces hT[17,4]; the
  17th row is ones (host-appended ones rows in c/w1), so row 16 of w2p —
  which the host sets to mlp_b2 + conv_w, both permuted — rides along and
  the result is directly the complete per-sample conv weight. The w2
  columns are round-robined over the four 32-partition k-groups in 512-col
  blocks so each tap's 8 matmuls spread across all 4 PE row-groups and run
  concurrently. Each tap's psum block is DMA-scattered (fp32, no engine
  cast) into wtmp[p][b*64+ci, t*64+co]; one wide [128, 576] tensor_copy
  per pair then downcasts to the bf16 conv weights wnew[p].
  Stage B (conv): host-padded bf16 x (130x130) for a sample pair lives as
  [ci(2 samples), h, w] across 128 partitions. The 128x128 PE runs in
  64x64 tiled mode as FOUR concurrent tiles — (sbuf rows, psum cols) =
  (0,0)/(64,0)/(0,64)/(64,64) = (sample a, top half)/(b, top)/(a, bottom)/
  (b, bottom) — each a full-utilization [K=64ci, M=64co, N=512] matmul, so
  no block-diagonal zero padding. Each psum tile [128, 512] accumulates 9
  shift-taps for one sample: partitions 0:64 = out rows h0:h0+4,
  partitions 64:128 = out rows h0+64:h0+68. Bias is added during the
  PSUM->SBUF copy (alternating DVE / scalar engines), then two half-tile
  DMAs store to DRAM on the evac engine's own queue.

  DMA queues: sync = consts + w2 + bulk x loads (sync engine is otherwise
  idle, so pushes issue immediately; x chunks are 13 padded rows, ordered
  top/bottom-half interleaved so conv group g's two image halves arrive
  together); gpsimd/vector = per-tap weight scatters (split by pair);
  scalar+vector = output stores, each after its own engine's evac.
"""

import sys

if "/opt/trn_rl_repo" not in sys.path:
    sys.path.insert(0, "/opt/trn_rl_repo")

import numpy as np
import ml_dtypes

B = 32
NCORES = 8
BPC = B // NCORES          # samples per core = 4
PAIRS = BPC // 2           # sample pairs per core = 2
CIN = COUT = 64
H = W = 128
H2 = H // 2                # bottom-half row offset = 64
HP = WP = 130              # padded dims
KH = KW = 3
NT = KH * KW               # taps = 9
CL = 8                     # c length
CL1 = CL + 1               # + ones row
MH = 16                    # mlp hidden
K2 = MH + 1                # mlp hidden + ones row
NGRP = H2 // 4             # psum groups per pair = 16 (4 rows x 2 halves each)
W2COL = (NT * CIN * COUT) // 4   # w2 columns per k-group = 9216
XCH = [0, 2, 1, 3, 4]      # 26-row x chunk load order (top/bottom interleave)
XROWS = 26                 # padded rows per x chunk (130 = 5 x 26)

_CACHE = {}


def _build():
    import concourse.bass as bass
    import concourse.mybir as mybir
    import concourse.tile as tile
    from concourse import bacc

    f32 = mybir.dt.float32
    bf16 = mybir.dt.bfloat16
    AF = mybir.ActivationFunctionType

    nc = bacc.Bacc("TRN2", target_bir_lowering=False, debug=False)

    xs_d = nc.dram_tensor("xsp", [BPC, CIN, HP * WP], bf16, kind="ExternalInput")
    ct_d = nc.dram_tensor("cT", [CL1, BPC], f32, kind="ExternalInput")
    w1_d = nc.dram_tensor("w1", [CL1, K2], f32, kind="ExternalInput")
    b1_d = nc.dram_tensor("b1", [K2, 1], f32, kind="ExternalInput")
    w2_d = nc.dram_tensor("w2p", [4, K2, W2COL], bf16, kind="ExternalInput")
    cb_d = nc.dram_tensor("cb2", [128, 1], f32, kind="ExternalInput")
    out_d = nc.dram_tensor("out", [BPC, COUT, H, W], f32, kind="ExternalOutput")

    with tile.TileContext(nc) as tc:
        with (
            tc.tile_pool(name="consts", bufs=1) as consts,
            tc.tile_pool(name="xpool", bufs=2) as xpool,
            tc.tile_pool(name="opool", bufs=6) as opool,
            tc.tile_pool(name="pspool", bufs=1, space=bass.MemorySpace.PSUM) as ps,
        ):
            # ---- constants in (sync queue; kept small + early) ----
            ct_sb = consts.tile([CL1, BPC], f32)
            nc.sync.dma_start(out=ct_sb[:], in_=ct_d.ap())
            w1_sb = consts.tile([CL1, K2], f32)
            nc.sync.dma_start(out=w1_sb[:], in_=w1_d.ap())
            b1_sb = consts.tile([K2, 1], f32)
            nc.sync.dma_start(out=b1_sb[:], in_=b1_d.ap())
            cb_sb = consts.tile([128, 1], f32)
            nc.sync.dma_start(out=cb_sb[:], in_=cb_d.ap())

            # w2 in 4 k-groups at partition strips 32g, spread over all 3
            # DMA queues (per-queue read bandwidth is the startup limiter)
            w2s = consts.tile([128, W2COL], bf16, name="w2s")
            w2_qs = [nc.sync, nc.gpsimd, nc.scalar, nc.gpsimd]
            for g in range(4):
                w2_qs[g].dma_start(
                    out=w2s[32 * g : 32 * g + K2, :], in_=w2_d.ap()[g]
                )

            # ---- bulk x loads: 26-padded-row chunks per pair, top/bottom
            # halves interleaved, alternating sync/scalar queues; all pushed
            # up front so both queues stream continuously ----
            xps = []
            for p in range(PAIRS):
                xp = xpool.tile([128, HP * WP], bf16, name=f"xp{p}", tag="xp")
                xps.append(xp)

            x_qs = [nc.sync, nc.scalar]
            xe = HP * WP // 5

            def load_x_chunk(p, i):
                k = XCH[i]
                x_qs[(len(XCH) * p + i) % 2].dma_start(
                    out=xps[p][:, k * xe : (k + 1) * xe],
                    in_=xs_d.ap()[2 * p : 2 * p + 2].rearrange(
                        "b c (k e) -> b c k e", e=xe
                    )[:, :, k, :],
                )

            # per-pair per-tap bf16 weight blocks [b*64+ci, t*64+co]
            wnew = []
            for p in range(PAIRS):
                wnew.append(consts.tile([128, NT * 64], bf16, name=f"wnew{p}"))

            for p in range(PAIRS):
                for i in range(len(XCH)):
                    load_x_chunk(p, i)

            # ---- stage A: conditioning MLP ----
            ph = ps.tile([K2, BPC], f32, tag="psA", bufs=2)
            nc.tensor.matmul(ph[:], w1_sb[:], ct_sb[:], start=True, stop=True)
            # hT replicated at partition offsets 0/32/64/96 to match the
            # packed w2 k-groups; replication DMAs ride the gpsimd queue
            # (short: w2 g1/g3 + scatters) so they aren't stuck behind x
            ht_sb = consts.tile([128, BPC], bf16)
            nc.scalar.activation(
                out=ht_sb[0:K2, :], in_=ph[:], func=AF.Relu, bias=b1_sb[:]
            )
            for g in range(1, 4):
                nc.gpsimd.dma_start(
                    out=ht_sb[32 * g : 32 * g + K2, :], in_=ht_sb[0:K2, :]
                )

            # adj[b, t, ci, co] = sum_k hT[k, b] w2p[k, t, ci, co]
            # (w2p row 16 carries mlp_b2 + conv_w, so adj is the full weight)
            # w2 512-col block i lives in k-group i%4 at cols (i//4)*512, so
            # each tap's 8 blocks run on all 4 PE row-groups concurrently.
            # psum->sbuf downcasts rotate over DVE/scalar/gpsimd; the
            # [2, 4096] -> [128, 64] tap scatters rotate over 4 DMA queues.
            def _cast_dve(out, in_):
                nc.vector.tensor_copy(out, in_)

            def _cast_act(out, in_):
                nc.scalar.activation(out=out, in_=in_, func=AF.Copy)

            cast_engs = [_cast_dve, _cast_act]
            # all taps' adj in one tile so pair-1 scatters can be deferred
            # to after pair-0's without blocking the cast pipeline
            adj = consts.tile([BPC, NT * CIN * COUT], bf16, name="adj")
            for t in range(NT):
                for i8 in range(8):
                    i = t * 8 + i8
                    g, col = i % 4, (i // 4) * 512
                    pa = ps.tile([BPC, 512], f32, tag="psA", bufs=2)
                    nc.tensor.matmul(
                        pa[:],
                        ht_sb[32 * g : 32 * g + K2, :],
                        w2s[32 * g : 32 * g + K2, col : col + 512],
                        start=True,
                        stop=True,
                        tile_position=(32 * g, 0),
                    )
                    cast_engs[i % 2](adj[:, i * 512 : (i + 1) * 512], pa[:])
                # pair-0 scatter right away (gpsimd queue only — the sync/
                # scalar queues have the bulk x loads queued ahead)
                nc.gpsimd.dma_start(
                    out=wnew[0][:, t * 64 : (t + 1) * 64],
                    in_=adj[0:2, t * CIN * COUT : (t + 1) * CIN * COUT],
                )
            for t in range(NT):
                nc.gpsimd.dma_start(
                    out=wnew[1][:, t * 64 : (t + 1) * 64],
                    in_=adj[2:4, t * CIN * COUT : (t + 1) * CIN * COUT],
                )

            # ---- stage B: per-pair conv in 64x64 PE tiled mode.
            # psum tile [128, 512] per sample: partitions 0:64 accumulate
            # out rows h0:h0+4, partitions 64:128 rows h0+64:h0+68; the
            # four (sample, half) tiles stream concurrently on the PE. ----
            store_qs = [nc.sync, nc.scalar, nc.gpsimd]
            for p in range(PAIRS):
                xp3 = xps[p].rearrange("p (h w) -> p h w", w=WP)
                for g in range(NGRP):
                    h0 = 4 * g
                    pss = [
                        ps.tile([128, 512], f32, tag="ps", bufs=6, name=f"po{p}_{g}_{s}")
                        for s in range(2)
                    ]
                    for t in range(NT):
                        kh, kw = divmod(t, 3)
                        r1 = h0 + kh
                        r2 = h0 + H2 + kh
                        for s in range(2):
                            wk = wnew[p][64 * s : 64 * s + 64, t * 64 : (t + 1) * 64]
                            nc.tensor.matmul(
                                pss[s][0:64, :],
                                wk,
                                xp3[64 * s : 64 * s + 64, r1 : r1 + 4, kw : kw + W],
                                start=(t == 0),
                                stop=(t == NT - 1),
                                tile_position=(64 * s, 0),
                            )
                            nc.tensor.matmul(
                                pss[s][64:128, :],
                                wk,
                                xp3[64 * s : 64 * s + 64, r2 : r2 + 4, kw : kw + W],
                                start=(t == 0),
                                stop=(t == NT - 1),
                                tile_position=(64 * s, 64),
                            )
                    for s in range(2):
                        os = opool.tile([128, 512], f32, name=f"os{p}_{g}_{s}", tag="os")
                        if (g + s) % 2 == 0:
                            nc.vector.tensor_scalar_add(os[:], pss[s][:], cb_sb[:])
                        else:
                            nc.scalar.activation(
                                out=os[:], in_=pss[s][:], func=AF.Identity,
                                bias=cb_sb[:],
                            )
                        b = 2 * p + s
                        q = store_qs[(2 * g + s) % 3]
                        q.dma_start(
                            out=out_d.ap()[b : b + 1, :, h0 : h0 + 4, :],
                            in_=os[0:64, :],
                        )
                        q.dma_start(
                            out=out_d.ap()[b : b + 1, :, h0 + H2 : h0 + H2 + 4, :],
                            in_=os[64:128, :],
                        )

    nc.compile()
    return nc


def _get_nc():
    if "nc" not in _CACHE:
        _CACHE["nc"] = _build()
    return _CACHE["nc"]


def _prep(x, c, conv_w, conv_b, mlp_w1, mlp_b1, mlp_w2, mlp_b2):
    x = np.ascontiguousarray(x, dtype=np.float32)
    c = np.ascontiguousarray(c, dtype=np.float32)
    conv_w = np.asarray(conv_w, dtype=np.float32)
    conv_b = np.asarray(conv_b, dtype=np.float32)
    mlp_w1 = np.asarray(mlp_w1, dtype=np.float32)
    mlp_b1 = np.asarray(mlp_b1, dtype=np.float32)
    mlp_w2 = np.asarray(mlp_w2, dtype=np.float32)
    mlp_b2 = np.asarray(mlp_b2, dtype=np.float32)

    # padded bf16 x, flattened spatial
    xsp = np.zeros((B, CIN, HP, WP), dtype=ml_dtypes.bfloat16)
    xsp[:, :, 1 : HP - 1, 1 : WP - 1] = x
    xsp = xsp.reshape(B, CIN, HP * WP)

    # w1' [CL1, K2]: [[w1, 0], [0, 1]]; cT' [CL1, BPC] gets a ones row
    w19 = np.zeros((CL1, K2), dtype=np.float32)
    w19[:CL, :MH] = mlp_w1
    w19[CL, MH] = 1.0
    b117 = np.concatenate([mlp_b1, np.zeros(1, np.float32)]).reshape(K2, 1)
    b117 = np.ascontiguousarray(b117, dtype=np.float32)

    # w2p[k, t, ci, co] = mlp_w2[k, co*576 + ci*9 + t]
    # row 16 = (mlp_b2 + conv_w), same permutation -> adj == full weight
    w2p = mlp_w2.reshape(MH, COUT, CIN, NT).transpose(0, 3, 2, 1)
    b2p = mlp_b2.reshape(COUT, CIN, NT).transpose(2, 1, 0)
    cwp = conv_w.reshape(COUT, CIN, NT).transpose(2, 1, 0)  # [t, ci, co]
    row16 = (b2p + cwp).reshape(1, -1)
    w2p = np.concatenate([w2p.reshape(MH, -1), row16], axis=0)
    # 512-col block i -> k-group i%4 at cols (i//4)*512 (round-robin so
    # each tap's 8 blocks hit all 4 PE row-groups)
    w2b = w2p.reshape(K2, 72, 512)
    w2g = np.zeros((4, K2, W2COL), dtype=np.float32)
    for i in range(72):
        w2g[i % 4, :, (i // 4) * 512 : (i // 4 + 1) * 512] = w2b[:, i, :]
    w2g = np.ascontiguousarray(w2g.astype(ml_dtypes.bfloat16))

    cb2 = np.ascontiguousarray(
        np.tile(conv_b.reshape(COUT, 1), (2, 1)), dtype=np.float32
    )

    in_maps = []
    for i in range(NCORES):
        sl = slice(i * BPC, (i + 1) * BPC)
        ct9 = np.concatenate([c[sl].T, np.ones((1, BPC), np.float32)], axis=0)
        in_maps.append(
            {
                "xsp": np.ascontiguousarray(xsp[sl]),
                "cT": np.ascontiguousarray(ct9),
                "w1": w19,
                "b1": b117,
                "w2p": w2g,
                "cb2": cb2,
            }
        )
    return in_maps


def _run(inputs, trace=False):
    from concourse.bass_utils import run_bass_kernel_spmd

    nc = _get_nc()
    in_maps = _prep(**inputs)
    res = run_bass_kernel_spmd(
        nc, in_maps, core_ids=list(range(NCORES)), trace=trace
    )
    out = np.concatenate([res.results[i]["out"] for i in range(NCORES)], axis=0)
    return out, res


def kernel(**inputs):
    out, _ = _run(inputs, trace=False)
    return out


# revision 23
# speedup vs baseline: 1.2165x; 1.2165x over previous
"""Trainium2 Bass kernel for conditional-adjustment conv (CAConv).

Per sample b: h = relu(c[b] @ mlp_w1 + mlp_b1); adj = h @ mlp_w2 + mlp_b2;
w[b] = conv_w + adj.reshape(Co,Ci,3,3); out[b] = conv2d(x[b], w[b], pad=1) + conv_b.

Sharding: data-parallel over batch, 4 samples per core on 8 cores (SPMD).

Per-core device kernel:
  Stage A (weight gen, bf16): one small matmul + relu produces hT[17,4]; the
  17th row is ones (host-appended ones rows in c/w1), so row 16 of w2p —
  which the host sets to mlp_b2 + conv_w, both permuted — rides along and
  the result is directly the complete per-sample conv weight. The w2
  columns are round-robined over the four 32-partition k-groups in 512-col
  blocks so each tap's 8 matmuls spread across all 4 PE row-groups and run
  concurrently. Each tap's psum block is DMA-scattered (fp32, no engine
  cast) into wtmp[p][b*64+ci, t*64+co]; one wide [128, 576] tensor_copy
  per pair then downcasts to the bf16 conv weights wnew[p].
  Stage B (conv): host-padded bf16 x (130x130) for a sample pair lives as
  [ci(2 samples), h, w] across 128 partitions. The 128x128 PE runs in
  64x64 tiled mode as FOUR concurrent tiles — (sbuf rows, psum cols) =
  (0,0)/(64,0)/(0,64)/(64,64) = (sample a, top half)/(b, top)/(a, bottom)/
  (b, bottom) — each a full-utilization [K=64ci, M=64co, N=512] matmul, so
  no block-diagonal zero padding. Each psum tile [128, 512] accumulates 9
  shift-taps for one sample: partitions 0:64 = out rows h0:h0+4,
  partitions 64:128 = out rows h0+64:h0+68. Bias is added during the
  PSUM->SBUF copy (alternating DVE / scalar engines), then two half-tile
  DMAs store to DRAM on the evac engine's own queue.

  DMA queues: sync = consts + w2 + bulk x loads (sync engine is otherwise
  idle, so pushes issue immediately; x chunks are 13 padded rows, ordered
  top/bottom-half interleaved so conv group g's two image halves arrive
  together); gpsimd/vector = per-tap weight scatters (split by pair);
  scalar+vector = output stores, each after its own engine's evac.
"""

import sys

if "/opt/trn_rl_repo" not in sys.path:
    sys.path.insert(0, "/opt/trn_rl_repo")

import numpy as np
import ml_dtypes

B = 32
NCORES = 8
BPC = B // NCORES          # samples per core = 4
PAIRS = BPC // 2           # sample pairs per core = 2
CIN = COUT = 64
H = W = 128
H2 = H // 2                # bottom-half row offset = 64
HP = WP = 130              # padded dims
KH = KW = 3
NT = KH * KW               # taps = 9
CL = 8                     # c length
CL1 = CL + 1               # + ones row
MH = 16                    # mlp hidden
K2 = MH + 1                # mlp hidden + ones row
NGRP = H2 // 4             # psum groups per pair = 16 (4 rows x 2 halves each)
W2COL = (NT * CIN * COUT) // 4   # w2 columns per k-group = 9216

_CACHE = {}


def _build():
    import concourse.bass as bass
    import concourse.mybir as mybir
    import concourse.tile as tile
    from concourse import bacc

    f32 = mybir.dt.float32
    bf16 = mybir.dt.bfloat16
    AF = mybir.ActivationFunctionType

    nc = bacc.Bacc("TRN2", target_bir_lowering=False, debug=False)

    xs_d = nc.dram_tensor("xsp", [BPC, CIN, HP * WP], bf16, kind="ExternalInput")
    ct_d = nc.dram_tensor("cT", [CL1, BPC], f32, kind="ExternalInput")
    w1_d = nc.dram_tensor("w1", [CL1, K2], f32, kind="ExternalInput")
    b1_d = nc.dram_tensor("b1", [K2, 1], f32, kind="ExternalInput")
    w2_d = nc.dram_tensor("w2p", [4, K2, W2COL], bf16, kind="ExternalInput")
    cb_d = nc.dram_tensor("cb2", [128, 1], f32, kind="ExternalInput")
    out_d = nc.dram_tensor("out", [BPC, COUT, H, W], f32, kind="ExternalOutput")

    with tile.TileContext(nc) as tc:
        with (
            tc.tile_pool(name="consts", bufs=1) as consts,
            tc.tile_pool(name="xpool", bufs=2) as xpool,
            tc.tile_pool(name="opool", bufs=6) as opool,
            tc.tile_pool(name="pspool", bufs=1, space=bass.MemorySpace.PSUM) as ps,
        ):
            # ---- constants in (sync queue; kept small + early) ----
            ct_sb = consts.tile([CL1, BPC], f32)
            nc.sync.dma_start(out=ct_sb[:], in_=ct_d.ap())
            w1_sb = consts.tile([CL1, K2], f32)
            nc.sync.dma_start(out=w1_sb[:], in_=w1_d.ap())
            b1_sb = consts.tile([K2, 1], f32)
            nc.sync.dma_start(out=b1_sb[:], in_=b1_d.ap())
            cb_sb = consts.tile([128, 1], f32)
            nc.sync.dma_start(out=cb_sb[:], in_=cb_d.ap())

            # w2 in 4 k-groups at partition strips 32g, ahead of x on the
            # two HWDGE queues
            w2s = consts.tile([128, W2COL], bf16, name="w2s")
            w2_qs = [nc.sync, nc.sync, nc.scalar, nc.scalar]
            for g in range(4):
                w2_qs[g].dma_start(
                    out=w2s[32 * g : 32 * g + K2, :], in_=w2_d.ap()[g]
                )

            # ---- bulk x loads: 2 half-image chunks per pair (top/bottom),
            # each pair's halves streaming on both HWDGE queues ----
            xps = []
            for p in range(PAIRS):
                xp = xpool.tile([128, HP * WP], bf16, name=f"xp{p}", tag="xp")
                xps.append(xp)

            xe = HP * WP // 2
            for p in range(PAIRS):
                for k in range(2):
                    [nc.sync, nc.scalar][k].dma_start(
                        out=xps[p][:, k * xe : (k + 1) * xe],
                        in_=xs_d.ap()[2 * p : 2 * p + 2].rearrange(
                            "b c (k e) -> b c k e", e=xe
                        )[:, :, k, :],
                    )

            # per-pair per-tap bf16 weight blocks [b*64+ci, t*64+co]
            wnew = []
            for p in range(PAIRS):
                wnew.append(consts.tile([128, NT * 64], bf16, name=f"wnew{p}"))

            # ---- stage A: conditioning MLP ----
            ph = ps.tile([K2, BPC], f32, tag="ps", bufs=8)
            nc.tensor.matmul(ph[:], w1_sb[:], ct_sb[:], start=True, stop=True)
            # hT replicated at partition offsets 0/32/64/96 to match the
            # packed w2 k-groups; replication DMAs ride the gpsimd queue
            # (short) so they aren't stuck behind the bulk loads
            ht_sb = consts.tile([128, BPC], bf16)
            nc.scalar.activation(
                out=ht_sb[0:K2, :], in_=ph[:], func=AF.Relu, bias=b1_sb[:]
            )
            for g in range(1, 4):
                nc.gpsimd.dma_start(
                    out=ht_sb[32 * g : 32 * g + K2, :], in_=ht_sb[0:K2, :]
                )

            def _cast_dve(out, in_):
                nc.vector.tensor_copy(out, in_)

            def _cast_act(out, in_):
                nc.scalar.activation(out=out, in_=in_, func=AF.Copy)

            cast_engs = [_cast_dve, _cast_act]
            # Stage-A matmuls write psum at partition strip 32g via
            # tile_position=(32g, 32g): one [128, 512] psum tile carries 4
            # k-groups' blocks, so the psum->sbuf downcast is full-width.
            # adj layout: partition 32g+b, col (ab*8 + cl)*576 + t*64 + co,
            # holding w2 512-block i = t*8 + ab*4 + g (ci = 32ab+8g+cl).
            adj = consts.tile([128, NT * CIN * COUT // 4], bf16, name="adj")
            adj_c = adj.rearrange("p (q cl tc) -> p q cl tc", q=2, cl=8)
            for t in range(NT):
                for ab in range(2):
                    pa = ps.tile([128, 512], f32, tag="ps", bufs=8)
                    for g in range(4):
                        nc.tensor.matmul(
                            pa[32 * g : 32 * g + BPC, :],
                            ht_sb[32 * g : 32 * g + K2, :],
                            w2s[32 * g : 32 * g + K2,
                                (2 * t + ab) * 512 : (2 * t + ab + 1) * 512],
                            start=True,
                            stop=True,
                            tile_position=(32 * g, 32 * g),
                        )
                    cast_engs[(2 * t + ab) % 2](
                        adj_c[:, ab, :, t * 64 : (t + 1) * 64],
                        pa.rearrange("p (cl co) -> p cl co", cl=8),
                    )
            # scatter all taps at once: per (pair, sample, ab, g) one DMA,
            # [1, 4608] contiguous -> [8 partitions, 576] runs into
            # wnew[p][64s+ci, t*64+co]
            for p in range(PAIRS):
                wn_s = wnew[p].rearrange(
                    "(s q g cl) f -> s q g cl f", s=2, q=2, g=4
                )
                for s in range(2):
                    for ab in range(2):
                        for g in range(4):
                            nc.gpsimd.dma_start(
                                out=wn_s[s, ab, g],
                                in_=adj[32 * g + 2 * p + s : 32 * g + 2 * p + s + 1,
                                        ab * 4608 : (ab + 1) * 4608],
                            )

            # ---- stage B: per-pair conv in 64x64 PE tiled mode.
            # psum tile [128, 512] per sample: partitions 0:64 accumulate
            # out rows h0:h0+4, partitions 64:128 rows h0+64:h0+68; the
            # four (sample, half) tiles stream concurrently on the PE. ----
            store_qs = [nc.sync, nc.scalar, nc.gpsimd]
            for p in range(PAIRS):
                xp3 = xps[p].rearrange("p (h w) -> p h w", w=WP)
                for g in range(NGRP):
                    h0 = 4 * g
                    pss = [
                        ps.tile([128, 512], f32, tag="ps", bufs=8, name=f"po{p}_{g}_{s}")
                        for s in range(2)
                    ]
                    for t in range(NT):
                        kh, kw = divmod(t, 3)
                        r1 = h0 + kh
                        r2 = h0 + H2 + kh
                        for s in range(2):
                            wk = wnew[p][64 * s : 64 * s + 64, t * 64 : (t + 1) * 64]
                            nc.tensor.matmul(
                                pss[s][0:64, :],
                                wk,
                                xp3[64 * s : 64 * s + 64, r1 : r1 + 4, kw : kw + W],
                                start=(t == 0),
                                stop=(t == NT - 1),
                                tile_position=(64 * s, 0),
                            )
                            nc.tensor.matmul(
                                pss[s][64:128, :],
                                wk,
                                xp3[64 * s : 64 * s + 64, r2 : r2 + 4, kw : kw + W],
                                start=(t == 0),
                                stop=(t == NT - 1),
                                tile_position=(64 * s, 64),
                            )
                    for s in range(2):
                        os = opool.tile([128, 512], f32, name=f"os{p}_{g}_{s}", tag="os")
                        if (g + s) % 2 == 0:
                            nc.vector.tensor_scalar_add(os[:], pss[s][:], cb_sb[:])
                        else:
                            nc.scalar.activation(
                                out=os[:], in_=pss[s][:], func=AF.Identity,
                                bias=cb_sb[:],
                            )
                        b = 2 * p + s
                        q = store_qs[(2 * g + s) % 3]
                        q.dma_start(
                            out=out_d.ap()[b : b + 1, :, h0 : h0 + 4, :],
                            in_=os[0:64, :],
                        )
                        q.dma_start(
                            out=out_d.ap()[b : b + 1, :, h0 + H2 : h0 + H2 + 4, :],
                            in_=os[64:128, :],
                        )

    nc.compile()
    return nc


def _get_nc():
    if "nc" not in _CACHE:
        _CACHE["nc"] = _build()
    return _CACHE["nc"]


def _prep(x, c, conv_w, conv_b, mlp_w1, mlp_b1, mlp_w2, mlp_b2):
    x = np.ascontiguousarray(x, dtype=np.float32)
    c = np.ascontiguousarray(c, dtype=np.float32)
    conv_w = np.asarray(conv_w, dtype=np.float32)
    conv_b = np.asarray(conv_b, dtype=np.float32)
    mlp_w1 = np.asarray(mlp_w1, dtype=np.float32)
    mlp_b1 = np.asarray(mlp_b1, dtype=np.float32)
    mlp_w2 = np.asarray(mlp_w2, dtype=np.float32)
    mlp_b2 = np.asarray(mlp_b2, dtype=np.float32)

    # padded bf16 x, flattened spatial
    xsp = np.zeros((B, CIN, HP, WP), dtype=ml_dtypes.bfloat16)
    xsp[:, :, 1 : HP - 1, 1 : WP - 1] = x
    xsp = xsp.reshape(B, CIN, HP * WP)

    # w1' [CL1, K2]: [[w1, 0], [0, 1]]; cT' [CL1, BPC] gets a ones row
    w19 = np.zeros((CL1, K2), dtype=np.float32)
    w19[:CL, :MH] = mlp_w1
    w19[CL, MH] = 1.0
    b117 = np.concatenate([mlp_b1, np.zeros(1, np.float32)]).reshape(K2, 1)
    b117 = np.ascontiguousarray(b117, dtype=np.float32)

    # w2p[k, t, ci, co] = mlp_w2[k, co*576 + ci*9 + t]
    # row 16 = (mlp_b2 + conv_w), same permutation -> adj == full weight
    w2p = mlp_w2.reshape(MH, COUT, CIN, NT).transpose(0, 3, 2, 1)
    b2p = mlp_b2.reshape(COUT, CIN, NT).transpose(2, 1, 0)
    cwp = conv_w.reshape(COUT, CIN, NT).transpose(2, 1, 0)  # [t, ci, co]
    row16 = (b2p + cwp).reshape(1, -1)
    w2p = np.concatenate([w2p.reshape(MH, -1), row16], axis=0)
    # 512-col block i -> k-group i%4 at cols (i//4)*512 (round-robin so
    # each tap's 8 blocks hit all 4 PE row-groups)
    w2b = w2p.reshape(K2, 72, 512)
    w2g = np.zeros((4, K2, W2COL), dtype=np.float32)
    for i in range(72):
        w2g[i % 4, :, (i // 4) * 512 : (i // 4 + 1) * 512] = w2b[:, i, :]
    w2g = np.ascontiguousarray(w2g.astype(ml_dtypes.bfloat16))

    cb2 = np.ascontiguousarray(
        np.tile(conv_b.reshape(COUT, 1), (2, 1)), dtype=np.float32
    )

    in_maps = []
    for i in range(NCORES):
        sl = slice(i * BPC, (i + 1) * BPC)
        ct9 = np.concatenate([c[sl].T, np.ones((1, BPC), np.float32)], axis=0)
        in_maps.append(
            {
                "xsp": np.ascontiguousarray(xsp[sl]),
                "cT": np.ascontiguousarray(ct9),
                "w1": w19,
                "b1": b117,
                "w2p": w2g,
                "cb2": cb2,
            }
        )
    return in_maps


def _run(inputs, trace=False):
    from concourse.bass_utils import run_bass_kernel_spmd

    nc = _get_nc()
    in_maps = _prep(**inputs)
    res = run_bass_kernel_spmd(
        nc, in_maps, core_ids=list(range(NCORES)), trace=trace
    )
    out = np.concatenate([res.results[i]["out"] for i in range(NCORES)], axis=0)
    return out, res


def kernel(**inputs):
    out, _ = _run(inputs, trace=False)
    return out


# revision 24
# speedup vs baseline: 1.2893x; 1.0598x over previous
"""Trainium2 Bass kernel for conditional-adjustment conv (CAConv).

Per sample b: h = relu(c[b] @ mlp_w1 + mlp_b1); adj = h @ mlp_w2 + mlp_b2;
w[b] = conv_w + adj.reshape(Co,Ci,3,3); out[b] = conv2d(x[b], w[b], pad=1) + conv_b.

Sharding: data-parallel over batch, 4 samples per core on 8 cores (SPMD).

Per-core device kernel:
  Stage A (weight gen, bf16): one small matmul + relu produces hT[17,4]; the
  17th row is ones (host-appended ones rows in c/w1), so row 16 of w2p —
  which the host sets to mlp_b2 + conv_w, both permuted — rides along and
  the result is directly the complete per-sample conv weight. The w2
  columns are round-robined over the four 32-partition k-groups in 512-col
  blocks so each tap's 8 matmuls spread across all 4 PE row-groups and run
  concurrently. Each tap's psum block is DMA-scattered (fp32, no engine
  cast) into wtmp[p][b*64+ci, t*64+co]; one wide [128, 576] tensor_copy
  per pair then downcasts to the bf16 conv weights wnew[p].
  Stage B (conv): host-padded bf16 x (130x130) for a sample pair lives as
  [ci(2 samples), h, w] across 128 partitions. The 128x128 PE runs in
  64x64 tiled mode as FOUR concurrent tiles — (sbuf rows, psum cols) =
  (0,0)/(64,0)/(0,64)/(64,64) = (sample a, top half)/(b, top)/(a, bottom)/
  (b, bottom) — each a full-utilization [K=64ci, M=64co, N=512] matmul, so
  no block-diagonal zero padding. Each psum tile [128, 512] accumulates 9
  shift-taps for one sample: partitions 0:64 = out rows h0:h0+4,
  partitions 64:128 = out rows h0+64:h0+68. Bias is added during the
  PSUM->SBUF copy (alternating DVE / scalar engines), then two half-tile
  DMAs store to DRAM on the evac engine's own queue.

  DMA queues: sync = consts + w2 + bulk x loads (sync engine is otherwise
  idle, so pushes issue immediately; x chunks are 13 padded rows, ordered
  top/bottom-half interleaved so conv group g's two image halves arrive
  together); gpsimd/vector = per-tap weight scatters (split by pair);
  scalar+vector = output stores, each after its own engine's evac.
"""

import sys

if "/opt/trn_rl_repo" not in sys.path:
    sys.path.insert(0, "/opt/trn_rl_repo")

import numpy as np
import ml_dtypes

B = 32
NCORES = 8
BPC = B // NCORES          # samples per core = 4
PAIRS = BPC // 2           # sample pairs per core = 2
CIN = COUT = 64
H = W = 128
H2 = H // 2                # bottom-half row offset = 64
HP = WP = 130              # padded dims
KH = KW = 3
NT = KH * KW               # taps = 9
CL = 8                     # c length
CL1 = CL + 1               # + ones row
MH = 16                    # mlp hidden
K2 = MH + 1                # mlp hidden + ones row
NGRP = H2 // 4             # psum groups per pair = 16 (4 rows x 2 halves each)
W2COL = (NT * CIN * COUT) // 4   # w2 columns per k-group = 9216

_CACHE = {}


def _build():
    import concourse.bass as bass
    import concourse.mybir as mybir
    import concourse.tile as tile
    from concourse import bacc

    f32 = mybir.dt.float32
    bf16 = mybir.dt.bfloat16
    AF = mybir.ActivationFunctionType

    nc = bacc.Bacc("TRN2", target_bir_lowering=False, debug=False)

    xs_d = nc.dram_tensor("xsp", [BPC, CIN, HP * WP], bf16, kind="ExternalInput")
    ct_d = nc.dram_tensor("cT", [CL1, BPC], f32, kind="ExternalInput")
    w1_d = nc.dram_tensor("w1", [CL1, K2], f32, kind="ExternalInput")
    b1_d = nc.dram_tensor("b1", [K2, 1], f32, kind="ExternalInput")
    w2_d = nc.dram_tensor("w2p", [4, K2, W2COL], bf16, kind="ExternalInput")
    cb_d = nc.dram_tensor("cb2", [128, 1], f32, kind="ExternalInput")
    out_d = nc.dram_tensor("out", [BPC, COUT, H, W], f32, kind="ExternalOutput")

    with tile.TileContext(nc) as tc:
        with (
            tc.tile_pool(name="consts", bufs=1) as consts,
            tc.tile_pool(name="xpool", bufs=2) as xpool,
            tc.tile_pool(name="opool", bufs=6) as opool,
            tc.tile_pool(name="pspool", bufs=1, space=bass.MemorySpace.PSUM) as ps,
        ):
            # ---- constants in (sync queue; kept small + early) ----
            ct_sb = consts.tile([CL1, BPC], f32)
            nc.sync.dma_start(out=ct_sb[:], in_=ct_d.ap())
            w1_sb = consts.tile([CL1, K2], f32)
            nc.sync.dma_start(out=w1_sb[:], in_=w1_d.ap())
            b1_sb = consts.tile([K2, 1], f32)
            nc.sync.dma_start(out=b1_sb[:], in_=b1_d.ap())
            cb_sb = consts.tile([128, 1], f32)
            nc.sync.dma_start(out=cb_sb[:], in_=cb_d.ap())

            # w2 in 4 k-groups at partition strips 32g, ahead of x on the
            # two HWDGE queues; 3 col-chunks per group keep descriptors
            # ~6KB so the queue round-robin stays fair (large descriptors
            # starve later transfers)
            w2s = consts.tile([128, W2COL], bf16, name="w2s")
            w2_qs = [nc.sync, nc.sync, nc.scalar, nc.scalar]
            w2e = W2COL // 3
            for g in range(4):
                for k in range(3):
                    w2_qs[g].dma_start(
                        out=w2s[32 * g : 32 * g + K2, k * w2e : (k + 1) * w2e],
                        in_=w2_d.ap()[g][:, k * w2e : (k + 1) * w2e],
                    )

            xps = []
            for p in range(PAIRS):
                xp = xpool.tile([128, HP * WP], bf16, name=f"xp{p}", tag="xp")
                xps.append(xp)

            # per-pair per-tap bf16 weight blocks [b*64+ci, t*64+co]
            wnew = []
            for p in range(PAIRS):
                wnew.append(consts.tile([128, NT * 64], bf16, name=f"wnew{p}"))

            # ---- stage A: conditioning MLP ----
            ph = ps.tile([K2, BPC], f32, tag="ps", bufs=8)
            nc.tensor.matmul(ph[:], w1_sb[:], ct_sb[:], start=True, stop=True)
            # hT replicated at partition offsets 0/32/64/96 to match the
            # packed w2 k-groups; replication DMAs ride the gpsimd queue
            # (short) so they aren't stuck behind the bulk loads
            ht_sb = consts.tile([128, BPC], bf16)
            nc.scalar.activation(
                out=ht_sb[0:K2, :], in_=ph[:], func=AF.Relu, bias=b1_sb[:]
            )
            for g in range(1, 4):
                nc.gpsimd.dma_start(
                    out=ht_sb[32 * g : 32 * g + K2, :], in_=ht_sb[0:K2, :]
                )

            # ---- bulk x loads: 26-padded-row chunks (6.76KB descriptors),
            # top/bottom-half interleaved so conv group g's two image
            # halves arrive together, alternating the two HWDGE queues ----
            xe = HP * WP // 5
            for p in range(PAIRS):
                for i, k in enumerate([0, 2, 1, 3, 4]):
                    [nc.sync, nc.scalar][(5 * p + i) % 2].dma_start(
                        out=xps[p][:, k * xe : (k + 1) * xe],
                        in_=xs_d.ap()[2 * p : 2 * p + 2].rearrange(
                            "b c (k e) -> b c k e", e=xe
                        )[:, :, k, :],
                    )

            def _cast_dve(out, in_):
                nc.vector.tensor_copy(out, in_)

            def _cast_act(out, in_):
                nc.scalar.activation(out=out, in_=in_, func=AF.Copy)

            cast_engs = [_cast_dve, _cast_act]
            # Stage-A matmuls write psum at partition strip 32g via
            # tile_position=(32g, 32g): one [128, 512] psum tile carries 4
            # k-groups' blocks, so the psum->sbuf downcast is full-width.
            # adj layout: partition 32g+b, col (ab*8 + cl)*576 + t*64 + co,
            # holding w2 512-block i = t*8 + ab*4 + g (ci = 32ab+8g+cl).
            adj = consts.tile([128, NT * CIN * COUT // 4], bf16, name="adj")
            adj_c = adj.rearrange("p (q cl tc) -> p q cl tc", q=2, cl=8)
            for t in range(NT):
                for ab in range(2):
                    pa = ps.tile([128, 512], f32, tag="ps", bufs=8)
                    for g in range(4):
                        nc.tensor.matmul(
                            pa[32 * g : 32 * g + BPC, :],
                            ht_sb[32 * g : 32 * g + K2, :],
                            w2s[32 * g : 32 * g + K2,
                                (2 * t + ab) * 512 : (2 * t + ab + 1) * 512],
                            start=True,
                            stop=True,
                            tile_position=(32 * g, 32 * g),
                        )
                    cast_engs[(2 * t + ab) % 2](
                        adj_c[:, ab, :, t * 64 : (t + 1) * 64],
                        pa.rearrange("p (cl co) -> p cl co", cl=8),
                    )
            # scatter all taps at once: per (pair, sample, ab, g) one DMA,
            # [1, 4608] contiguous -> [8 partitions, 576] runs into
            # wnew[p][64s+ci, t*64+co]
            for p in range(PAIRS):
                wn_s = wnew[p].rearrange(
                    "(s q g cl) f -> s q g cl f", s=2, q=2, g=4
                )
                for s in range(2):
                    for ab in range(2):
                        for g in range(4):
                            nc.gpsimd.dma_start(
                                out=wn_s[s, ab, g],
                                in_=adj[32 * g + 2 * p + s : 32 * g + 2 * p + s + 1,
                                        ab * 4608 : (ab + 1) * 4608],
                            )

            # ---- stage B: per-pair conv in 64x64 PE tiled mode.
            # psum tile [128, 512] per sample: partitions 0:64 accumulate
            # out rows h0:h0+4, partitions 64:128 rows h0+64:h0+68; the
            # four (sample, half) tiles stream concurrently on the PE. ----
            store_qs = [nc.sync, nc.scalar, nc.gpsimd]
            for p in range(PAIRS):
                xp3 = xps[p].rearrange("p (h w) -> p h w", w=WP)
                for g in range(NGRP):
                    h0 = 4 * g
                    pss = [
                        ps.tile([128, 512], f32, tag="ps", bufs=8, name=f"po{p}_{g}_{s}")
                        for s in range(2)
                    ]
                    for t in range(NT):
                        kh, kw = divmod(t, 3)
                        r1 = h0 + kh
                        r2 = h0 + H2 + kh
                        for s in range(2):
                            wk = wnew[p][64 * s : 64 * s + 64, t * 64 : (t + 1) * 64]
                            nc.tensor.matmul(
                                pss[s][0:64, :],
                                wk,
                                xp3[64 * s : 64 * s + 64, r1 : r1 + 4, kw : kw + W],
                                start=(t == 0),
                                stop=(t == NT - 1),
                                tile_position=(64 * s, 0),
                            )
                            nc.tensor.matmul(
                                pss[s][64:128, :],
                                wk,
                                xp3[64 * s : 64 * s + 64, r2 : r2 + 4, kw : kw + W],
                                start=(t == 0),
                                stop=(t == NT - 1),
                                tile_position=(64 * s, 64),
                            )
                    for s in range(2):
                        os = opool.tile([128, 512], f32, name=f"os{p}_{g}_{s}", tag="os")
                        if (g + s) % 2 == 0:
                            nc.vector.tensor_scalar_add(os[:], pss[s][:], cb_sb[:])
                        else:
                            nc.scalar.activation(
                                out=os[:], in_=pss[s][:], func=AF.Identity,
                                bias=cb_sb[:],
                            )
                        b = 2 * p + s
                        q = store_qs[(2 * g + s) % 3]
                        q.dma_start(
                            out=out_d.ap()[b : b + 1, :, h0 : h0 + 4, :],
                            in_=os[0:64, :],
                        )
                        q.dma_start(
                            out=out_d.ap()[b : b + 1, :, h0 + H2 : h0 + H2 + 4, :],
                            in_=os[64:128, :],
                        )

    nc.compile()
    return nc


def _get_nc():
    if "nc" not in _CACHE:
        _CACHE["nc"] = _build()
    return _CACHE["nc"]


def _prep(x, c, conv_w, conv_b, mlp_w1, mlp_b1, mlp_w2, mlp_b2):
    x = np.ascontiguousarray(x, dtype=np.float32)
    c = np.ascontiguousarray(c, dtype=np.float32)
    conv_w = np.asarray(conv_w, dtype=np.float32)
    conv_b = np.asarray(conv_b, dtype=np.float32)
    mlp_w1 = np.asarray(mlp_w1, dtype=np.float32)
    mlp_b1 = np.asarray(mlp_b1, dtype=np.float32)
    mlp_w2 = np.asarray(mlp_w2, dtype=np.float32)
    mlp_b2 = np.asarray(mlp_b2, dtype=np.float32)

    # padded bf16 x, flattened spatial
    xsp = np.zeros((B, CIN, HP, WP), dtype=ml_dtypes.bfloat16)
    xsp[:, :, 1 : HP - 1, 1 : WP - 1] = x
    xsp = xsp.reshape(B, CIN, HP * WP)

    # w1' [CL1, K2]: [[w1, 0], [0, 1]]; cT' [CL1, BPC] gets a ones row
    w19 = np.zeros((CL1, K2), dtype=np.float32)
    w19[:CL, :MH] = mlp_w1
    w19[CL, MH] = 1.0
    b117 = np.concatenate([mlp_b1, np.zeros(1, np.float32)]).reshape(K2, 1)
    b117 = np.ascontiguousarray(b117, dtype=np.float32)

    # w2p[k, t, ci, co] = mlp_w2[k, co*576 + ci*9 + t]
    # row 16 = (mlp_b2 + conv_w), same permutation -> adj == full weight
    w2p = mlp_w2.reshape(MH, COUT, CIN, NT).transpose(0, 3, 2, 1)
    b2p = mlp_b2.reshape(COUT, CIN, NT).transpose(2, 1, 0)
    cwp = conv_w.reshape(COUT, CIN, NT).transpose(2, 1, 0)  # [t, ci, co]
    row16 = (b2p + cwp).reshape(1, -1)
    w2p = np.concatenate([w2p.reshape(MH, -1), row16], axis=0)
    # 512-col block i -> k-group i%4 at cols (i//4)*512 (round-robin so
    # each tap's 8 blocks hit all 4 PE row-groups)
    w2b = w2p.reshape(K2, 72, 512)
    w2g = np.zeros((4, K2, W2COL), dtype=np.float32)
    for i in range(72):
        w2g[i % 4, :, (i // 4) * 512 : (i // 4 + 1) * 512] = w2b[:, i, :]
    w2g = np.ascontiguousarray(w2g.astype(ml_dtypes.bfloat16))

    cb2 = np.ascontiguousarray(
        np.tile(conv_b.reshape(COUT, 1), (2, 1)), dtype=np.float32
    )

    in_maps = []
    for i in range(NCORES):
        sl = slice(i * BPC, (i + 1) * BPC)
        ct9 = np.concatenate([c[sl].T, np.ones((1, BPC), np.float32)], axis=0)
        in_maps.append(
            {
                "xsp": np.ascontiguousarray(xsp[sl]),
                "cT": np.ascontiguousarray(ct9),
                "w1": w19,
                "b1": b117,
                "w2p": w2g,
                "cb2": cb2,
            }
        )
    return in_maps


def _run(inputs, trace=False):
    from concourse.bass_utils import run_bass_kernel_spmd

    nc = _get_nc()
    in_maps = _prep(**inputs)
    res = run_bass_kernel_spmd(
        nc, in_maps, core_ids=list(range(NCORES)), trace=trace
    )
    out = np.concatenate([res.results[i]["out"] for i in range(NCORES)], axis=0)
    return out, res


def kernel(**inputs):
    out, _ = _run(inputs, trace=False)
    return out


# revision 25
# speedup vs baseline: 1.3163x; 1.0210x over previous
"""Trainium2 Bass kernel for conditional-adjustment conv (CAConv).

Per sample b: h = relu(c[b] @ mlp_w1 + mlp_b1); adj = h @ mlp_w2 + mlp_b2;
w[b] = conv_w + adj.reshape(Co,Ci,3,3); out[b] = conv2d(x[b], w[b], pad=1) + conv_b.

Sharding: data-parallel over batch, 4 samples per core on 8 cores (SPMD).

Per-core device kernel:
  Stage A (weight gen, bf16): one small matmul + relu produces hT[17,4]; the
  17th row is ones (host-appended ones rows in c/w1), so row 16 of w2p —
  which the host sets to mlp_b2 + conv_w, both permuted — rides along and
  the result is directly the complete per-sample conv weight. The w2
  columns are round-robined over the four 32-partition k-groups in 512-col
  blocks so each tap's 8 matmuls spread across all 4 PE row-groups and run
  concurrently. Each tap's psum block is DMA-scattered (fp32, no engine
  cast) into wtmp[p][b*64+ci, t*64+co]; one wide [128, 576] tensor_copy
  per pair then downcasts to the bf16 conv weights wnew[p].
  Stage B (conv): host-padded bf16 x (130x130) for a sample pair lives as
  [ci(2 samples), h, w] across 128 partitions. The 128x128 PE runs in
  64x64 tiled mode as FOUR concurrent tiles — (sbuf rows, psum cols) =
  (0,0)/(64,0)/(0,64)/(64,64) = (sample a, top half)/(b, top)/(a, bottom)/
  (b, bottom) — each a full-utilization [K=64ci, M=64co, N=512] matmul, so
  no block-diagonal zero padding. Each psum tile [128, 512] accumulates 9
  shift-taps for one sample: partitions 0:64 = out rows h0:h0+4,
  partitions 64:128 = out rows h0+64:h0+68. Bias is added during the
  PSUM->SBUF copy (alternating DVE / scalar engines), then two half-tile
  DMAs store to DRAM on the evac engine's own queue.

  DMA queues: sync = consts + w2 + bulk x loads (sync engine is otherwise
  idle, so pushes issue immediately; x chunks are 13 padded rows, ordered
  top/bottom-half interleaved so conv group g's two image halves arrive
  together); gpsimd/vector = per-tap weight scatters (split by pair);
  scalar+vector = output stores, each after its own engine's evac.
"""

import sys

if "/opt/trn_rl_repo" not in sys.path:
    sys.path.insert(0, "/opt/trn_rl_repo")

import numpy as np
import ml_dtypes

B = 32
NCORES = 8
BPC = B // NCORES          # samples per core = 4
PAIRS = BPC // 2           # sample pairs per core = 2
CIN = COUT = 64
H = W = 128
H2 = H // 2                # bottom-half row offset = 64
HP = WP = 130              # padded dims
KH = KW = 3
NT = KH * KW               # taps = 9
CL = 8                     # c length
CL1 = CL + 1               # + ones row
MH = 16                    # mlp hidden
K2 = MH + 1                # mlp hidden + ones row
NGRP = H2 // 4             # psum groups per pair = 16 (4 rows x 2 halves each)
W2COL = (NT * CIN * COUT) // 4   # w2 columns per k-group = 9216

_CACHE = {}


def _build():
    import concourse.bass as bass
    import concourse.mybir as mybir
    import concourse.tile as tile
    from concourse import bacc

    f32 = mybir.dt.float32
    bf16 = mybir.dt.bfloat16
    AF = mybir.ActivationFunctionType

    nc = bacc.Bacc("TRN2", target_bir_lowering=False, debug=False)

    xs_d = nc.dram_tensor("xsp", [BPC, CIN, HP * WP], bf16, kind="ExternalInput")
    ct_d = nc.dram_tensor("cT", [CL1, BPC], f32, kind="ExternalInput")
    w1_d = nc.dram_tensor("w1", [CL1, K2], f32, kind="ExternalInput")
    b1_d = nc.dram_tensor("b1", [K2, 1], f32, kind="ExternalInput")
    w2_d = nc.dram_tensor("w2p", [4, K2, W2COL], bf16, kind="ExternalInput")
    cb_d = nc.dram_tensor("cb2", [128, 1], f32, kind="ExternalInput")
    out_d = nc.dram_tensor("out", [BPC, COUT, H, W], f32, kind="ExternalOutput")

    with tile.TileContext(nc) as tc:
        with (
            tc.tile_pool(name="consts", bufs=1) as consts,
            tc.tile_pool(name="xpool", bufs=2) as xpool,
            tc.tile_pool(name="opool", bufs=6) as opool,
            tc.tile_pool(name="pspool", bufs=1, space=bass.MemorySpace.PSUM) as ps,
        ):
            # ---- constants in (sync queue; kept small + early) ----
            ct_sb = consts.tile([CL1, BPC], f32)
            nc.sync.dma_start(out=ct_sb[:], in_=ct_d.ap())
            w1_sb = consts.tile([CL1, K2], f32)
            nc.sync.dma_start(out=w1_sb[:], in_=w1_d.ap())
            b1_sb = consts.tile([K2, 1], f32)
            nc.sync.dma_start(out=b1_sb[:], in_=b1_d.ap())
            cb_sb = consts.tile([128, 1], f32)
            nc.sync.dma_start(out=cb_sb[:], in_=cb_d.ap())

            # w2 in 4 k-groups at partition strips 32g, ahead of x on the
            # two HWDGE queues; 3 col-chunks per group keep descriptors
            # ~6KB so the queue round-robin stays fair (large descriptors
            # starve later transfers)
            w2s = consts.tile([128, W2COL], bf16, name="w2s")
            w2_qs = [nc.sync, nc.sync, nc.scalar, nc.scalar]
            w2e = W2COL // 3
            for k in range(3):
                for g in range(4):
                    w2_qs[g].dma_start(
                        out=w2s[32 * g : 32 * g + K2, k * w2e : (k + 1) * w2e],
                        in_=w2_d.ap()[g][:, k * w2e : (k + 1) * w2e],
                    )

            xps = []
            for p in range(PAIRS):
                xp = xpool.tile([128, HP * WP], bf16, name=f"xp{p}", tag="xp")
                xps.append(xp)

            # per-pair per-tap bf16 weight blocks [b*64+ci, t*64+co]
            wnew = []
            for p in range(PAIRS):
                wnew.append(consts.tile([128, NT * 64], bf16, name=f"wnew{p}"))

            # ---- stage A: conditioning MLP ----
            ph = ps.tile([K2, BPC], f32, tag="ps", bufs=8)
            nc.tensor.matmul(ph[:], w1_sb[:], ct_sb[:], start=True, stop=True)
            # hT replicated at partition offsets 0/32/64/96 to match the
            # packed w2 k-groups; replication DMAs ride the gpsimd queue
            # (short) so they aren't stuck behind the bulk loads
            ht_sb = consts.tile([128, BPC], bf16)
            nc.scalar.activation(
                out=ht_sb[0:K2, :], in_=ph[:], func=AF.Relu, bias=b1_sb[:]
            )
            for g in range(1, 4):
                nc.gpsimd.dma_start(
                    out=ht_sb[32 * g : 32 * g + K2, :], in_=ht_sb[0:K2, :]
                )

            # ---- bulk x loads: 26-padded-row chunks (6.76KB descriptors),
            # top/bottom-half interleaved so conv group g's two image
            # halves arrive together, alternating the two HWDGE queues ----
            xe = HP * WP // 5
            for p in range(PAIRS):
                for i, k in enumerate([0, 2, 1, 3, 4]):
                    [nc.sync, nc.scalar][(5 * p + i) % 2].dma_start(
                        out=xps[p][:, k * xe : (k + 1) * xe],
                        in_=xs_d.ap()[2 * p : 2 * p + 2].rearrange(
                            "b c (k e) -> b c k e", e=xe
                        )[:, :, k, :],
                    )

            def _cast_dve(out, in_):
                nc.vector.tensor_copy(out, in_)

            def _cast_act(out, in_):
                nc.scalar.activation(out=out, in_=in_, func=AF.Copy)

            cast_engs = [_cast_dve, _cast_act]
            # Stage-A matmuls write psum at partition strip 32g via
            # tile_position=(32g, 32g): one [128, 512] psum tile carries 4
            # k-groups' blocks, so the psum->sbuf downcast is full-width.
            # adj layout: partition 32g+b, col (ab*8 + cl)*576 + t*64 + co,
            # holding w2 512-block i = t*8 + ab*4 + g (ci = 32ab+8g+cl).
            adj = consts.tile([128, NT * CIN * COUT // 4], bf16, name="adj")
            adj_c = adj.rearrange("p (q cl tc) -> p q cl tc", q=2, cl=8)
            for t in range(NT):
                for ab in range(2):
                    pa = ps.tile([128, 512], f32, tag="ps", bufs=8)
                    for g in range(4):
                        nc.tensor.matmul(
                            pa[32 * g : 32 * g + BPC, :],
                            ht_sb[32 * g : 32 * g + K2, :],
                            w2s[32 * g : 32 * g + K2,
                                (2 * t + ab) * 512 : (2 * t + ab + 1) * 512],
                            start=True,
                            stop=True,
                            tile_position=(32 * g, 32 * g),
                        )
                    cast_engs[(2 * t + ab) % 2](
                        adj_c[:, ab, :, t * 64 : (t + 1) * 64],
                        pa.rearrange("p (cl co) -> p cl co", cl=8),
                    )
            # scatter all taps at once: per (pair, sample, ab, g) one DMA,
            # [1, 4608] contiguous -> [8 partitions, 576] runs into
            # wnew[p][64s+ci, t*64+co]
            for p in range(PAIRS):
                wn_s = wnew[p].rearrange(
                    "(s q g cl) f -> s q g cl f", s=2, q=2, g=4
                )
                for s in range(2):
                    for ab in range(2):
                        for g in range(4):
                            nc.gpsimd.dma_start(
                                out=wn_s[s, ab, g],
                                in_=adj[32 * g + 2 * p + s : 32 * g + 2 * p + s + 1,
                                        ab * 4608 : (ab + 1) * 4608],
                            )

            # ---- stage B: per-pair conv in 64x64 PE tiled mode.
            # psum tile [128, 512] per sample: partitions 0:64 accumulate
            # out rows h0:h0+4, partitions 64:128 rows h0+64:h0+68; the
            # four (sample, half) tiles stream concurrently on the PE. ----
            store_qs = [nc.sync, nc.scalar, nc.gpsimd]
            for p in range(PAIRS):
                xp3 = xps[p].rearrange("p (h w) -> p h w", w=WP)
                for g in range(NGRP):
                    h0 = 4 * g
                    pss = [
                        ps.tile([128, 512], f32, tag="ps", bufs=8, name=f"po{p}_{g}_{s}")
                        for s in range(2)
                    ]
                    for t in range(NT):
                        kh, kw = divmod(t, 3)
                        r1 = h0 + kh
                        r2 = h0 + H2 + kh
                        for s in range(2):
                            wk = wnew[p][64 * s : 64 * s + 64, t * 64 : (t + 1) * 64]
                            nc.tensor.matmul(
                                pss[s][0:64, :],
                                wk,
                                xp3[64 * s : 64 * s + 64, r1 : r1 + 4, kw : kw + W],
                                start=(t == 0),
                                stop=(t == NT - 1),
                                tile_position=(64 * s, 0),
                            )
                            nc.tensor.matmul(
                                pss[s][64:128, :],
                                wk,
                                xp3[64 * s : 64 * s + 64, r2 : r2 + 4, kw : kw + W],
                                start=(t == 0),
                                stop=(t == NT - 1),
                                tile_position=(64 * s, 64),
                            )
                    for s in range(2):
                        os = opool.tile([128, 512], f32, name=f"os{p}_{g}_{s}", tag="os")
                        if (g + s) % 2 == 0:
                            nc.vector.tensor_scalar_add(os[:], pss[s][:], cb_sb[:])
                        else:
                            nc.scalar.activation(
                                out=os[:], in_=pss[s][:], func=AF.Identity,
                                bias=cb_sb[:],
                            )
                        b = 2 * p + s
                        q = store_qs[(2 * g + s) % 3]
                        q.dma_start(
                            out=out_d.ap()[b : b + 1, :, h0 : h0 + 4, :],
                            in_=os[0:64, :],
                        )
                        q.dma_start(
                            out=out_d.ap()[b : b + 1, :, h0 + H2 : h0 + H2 + 4, :],
                            in_=os[64:128, :],
                        )

    nc.compile()
    return nc


def _get_nc():
    if "nc" not in _CACHE:
        _CACHE["nc"] = _build()
    return _CACHE["nc"]


def _prep(x, c, conv_w, conv_b, mlp_w1, mlp_b1, mlp_w2, mlp_b2):
    x = np.ascontiguousarray(x, dtype=np.float32)
    c = np.ascontiguousarray(c, dtype=np.float32)
    conv_w = np.asarray(conv_w, dtype=np.float32)
    conv_b = np.asarray(conv_b, dtype=np.float32)
    mlp_w1 = np.asarray(mlp_w1, dtype=np.float32)
    mlp_b1 = np.asarray(mlp_b1, dtype=np.float32)
    mlp_w2 = np.asarray(mlp_w2, dtype=np.float32)
    mlp_b2 = np.asarray(mlp_b2, dtype=np.float32)

    # padded bf16 x, flattened spatial
    xsp = np.zeros((B, CIN, HP, WP), dtype=ml_dtypes.bfloat16)
    xsp[:, :, 1 : HP - 1, 1 : WP - 1] = x
    xsp = xsp.reshape(B, CIN, HP * WP)

    # w1' [CL1, K2]: [[w1, 0], [0, 1]]; cT' [CL1, BPC] gets a ones row
    w19 = np.zeros((CL1, K2), dtype=np.float32)
    w19[:CL, :MH] = mlp_w1
    w19[CL, MH] = 1.0
    b117 = np.concatenate([mlp_b1, np.zeros(1, np.float32)]).reshape(K2, 1)
    b117 = np.ascontiguousarray(b117, dtype=np.float32)

    # w2p[k, t, ci, co] = mlp_w2[k, co*576 + ci*9 + t]
    # row 16 = (mlp_b2 + conv_w), same permutation -> adj == full weight
    w2p = mlp_w2.reshape(MH, COUT, CIN, NT).transpose(0, 3, 2, 1)
    b2p = mlp_b2.reshape(COUT, CIN, NT).transpose(2, 1, 0)
    cwp = conv_w.reshape(COUT, CIN, NT).transpose(2, 1, 0)  # [t, ci, co]
    row16 = (b2p + cwp).reshape(1, -1)
    w2p = np.concatenate([w2p.reshape(MH, -1), row16], axis=0)
    # 512-col block i -> k-group i%4 at cols (i//4)*512 (round-robin so
    # each tap's 8 blocks hit all 4 PE row-groups)
    w2b = w2p.reshape(K2, 72, 512)
    w2g = np.zeros((4, K2, W2COL), dtype=np.float32)
    for i in range(72):
        w2g[i % 4, :, (i // 4) * 512 : (i // 4 + 1) * 512] = w2b[:, i, :]
    w2g = np.ascontiguousarray(w2g.astype(ml_dtypes.bfloat16))

    cb2 = np.ascontiguousarray(
        np.tile(conv_b.reshape(COUT, 1), (2, 1)), dtype=np.float32
    )

    in_maps = []
    for i in range(NCORES):
        sl = slice(i * BPC, (i + 1) * BPC)
        ct9 = np.concatenate([c[sl].T, np.ones((1, BPC), np.float32)], axis=0)
        in_maps.append(
            {
                "xsp": np.ascontiguousarray(xsp[sl]),
                "cT": np.ascontiguousarray(ct9),
                "w1": w19,
                "b1": b117,
                "w2p": w2g,
                "cb2": cb2,
            }
        )
    return in_maps


def _run(inputs, trace=False):
    from concourse.bass_utils import run_bass_kernel_spmd

    nc = _get_nc()
    in_maps = _prep(**inputs)
    res = run_bass_kernel_spmd(
        nc, in_maps, core_ids=list(range(NCORES)), trace=trace
    )
    out = np.concatenate([res.results[i]["out"] for i in range(NCORES)], axis=0)
    return out, res


def kernel(**inputs):
    out, _ = _run(inputs, trace=False)
    return out


# revision 26
# speedup vs baseline: 1.6876x; 1.2821x over previous
"""Trainium2 Bass kernel for conditional-adjustment conv (CAConv).

Per sample b: h = relu(c[b] @ mlp_w1 + mlp_b1); adj = h @ mlp_w2 + mlp_b2;
w[b] = conv_w + adj.reshape(Co,Ci,3,3); out[b] = conv2d(x[b], w[b], pad=1) + conv_b.

Sharding: data-parallel over batch, 4 samples per core on 8 cores (SPMD).

Per-core device kernel:
  Stage A (weight gen, bf16): one small matmul + relu produces hT[17,4]; the
  17th row is ones (host-appended ones rows in c/w1), so row 16 of w2p —
  which the host sets to mlp_b2 + conv_w, both permuted — rides along and
  the result is directly the complete per-sample conv weight. The w2
  columns are round-robined over the four 32-partition k-groups in 512-col
  blocks so each tap's 8 matmuls spread across all 4 PE row-groups and run
  concurrently. Each tap's psum block is DMA-scattered (fp32, no engine
  cast) into wtmp[p][b*64+ci, t*64+co]; one wide [128, 576] tensor_copy
  per pair then downcasts to the bf16 conv weights wnew[p].
  Stage B (conv): host-padded bf16 x (130x130) for a sample pair lives as
  [ci(2 samples), h, w] across 128 partitions. The 128x128 PE runs in
  64x64 tiled mode as FOUR concurrent tiles — (sbuf rows, psum cols) =
  (0,0)/(64,0)/(0,64)/(64,64) = (sample a, top half)/(b, top)/(a, bottom)/
  (b, bottom) — each a full-utilization [K=64ci, M=64co, N=512] matmul, so
  no block-diagonal zero padding. Each psum tile [128, 512] accumulates 9
  shift-taps for one sample: partitions 0:64 = out rows h0:h0+4,
  partitions 64:128 = out rows h0+64:h0+68. Bias is added during the
  PSUM->SBUF copy (alternating DVE / scalar engines), then two half-tile
  DMAs store to DRAM on the evac engine's own queue.

  DMA queues: sync = consts + w2 + bulk x loads (sync engine is otherwise
  idle, so pushes issue immediately; x chunks are 13 padded rows, ordered
  top/bottom-half interleaved so conv group g's two image halves arrive
  together); gpsimd/vector = per-tap weight scatters (split by pair);
  scalar+vector = output stores, each after its own engine's evac.
"""

import sys

if "/opt/trn_rl_repo" not in sys.path:
    sys.path.insert(0, "/opt/trn_rl_repo")

import numpy as np
import ml_dtypes

B = 32
NCORES = 8
BPC = B // NCORES          # samples per core = 4
PAIRS = BPC // 2           # sample pairs per core = 2
CIN = COUT = 64
H = W = 128
H2 = H // 2                # bottom-half row offset = 64
HP = WP = 130              # padded dims
KH = KW = 3
NT = KH * KW               # taps = 9
CL = 8                     # c length
CL1 = CL + 1               # + ones row
MH = 16                    # mlp hidden
K2 = MH + 1                # mlp hidden + ones row
NGRP = H2 // 4             # psum groups per pair = 16 (4 rows x 2 halves each)
W2COL = (NT * CIN * COUT) // 4   # w2 columns per k-group = 9216

_CACHE = {}


def _build():
    import concourse.bass as bass
    import concourse.mybir as mybir
    import concourse.tile as tile
    from concourse import bacc

    f32 = mybir.dt.float32
    bf16 = mybir.dt.bfloat16
    AF = mybir.ActivationFunctionType

    nc = bacc.Bacc("TRN2", target_bir_lowering=False, debug=False)

    xs_d = nc.dram_tensor("xsp", [BPC, CIN, HP * WP], bf16, kind="ExternalInput")
    ct_d = nc.dram_tensor("cT", [CL1, BPC], f32, kind="ExternalInput")
    w1_d = nc.dram_tensor("w1", [CL1, K2], f32, kind="ExternalInput")
    b1_d = nc.dram_tensor("b1", [K2, 1], f32, kind="ExternalInput")
    w2_d = nc.dram_tensor("w2p", [128, W2COL], bf16, kind="ExternalInput")
    cb_d = nc.dram_tensor("cb2", [128, 1], f32, kind="ExternalInput")
    out_d = nc.dram_tensor("out", [BPC, COUT, H, W], f32, kind="ExternalOutput")

    with tile.TileContext(nc) as tc:
        with (
            tc.tile_pool(name="consts", bufs=1) as consts,
            tc.tile_pool(name="xpool", bufs=2) as xpool,
            tc.tile_pool(name="opool", bufs=6) as opool,
            tc.tile_pool(name="pspool", bufs=1, space=bass.MemorySpace.PSUM) as ps,
        ):
            # ---- constants in (sync queue; kept small + early) ----
            ct_sb = consts.tile([CL1, BPC], f32)
            nc.sync.dma_start(out=ct_sb[:], in_=ct_d.ap())
            w1_sb = consts.tile([CL1, K2], f32)
            nc.sync.dma_start(out=w1_sb[:], in_=w1_d.ap())
            b1_sb = consts.tile([K2, 1], f32)
            nc.sync.dma_start(out=b1_sb[:], in_=b1_d.ap())
            cb_sb = consts.tile([128, 1], f32)
            nc.sync.dma_start(out=cb_sb[:], in_=cb_d.ap())

            # w2 host-padded to all 128 partitions (k-strips at 32g+k,
            # dead rows zero): full-width loads run ~10x faster than
            # 17-partition strip loads (per-partition write rate limit),
            # well worth the extra 1.1MB of dead-row traffic
            w2s = consts.tile([128, W2COL], bf16, name="w2s")
            w2e = W2COL // 4
            for k in range(4):
                [nc.sync, nc.scalar][k % 2].dma_start(
                    out=w2s[:, k * w2e : (k + 1) * w2e],
                    in_=w2_d.ap()[:, k * w2e : (k + 1) * w2e],
                )

            xps = []
            for p in range(PAIRS):
                xp = xpool.tile([128, HP * WP], bf16, name=f"xp{p}", tag="xp")
                xps.append(xp)

            # per-pair per-tap bf16 weight blocks [b*64+ci, t*64+co]
            wnew = []
            for p in range(PAIRS):
                wnew.append(consts.tile([128, NT * 64], bf16, name=f"wnew{p}"))

            # ---- stage A: conditioning MLP ----
            ph = ps.tile([K2, BPC], f32, tag="ps", bufs=8)
            nc.tensor.matmul(ph[:], w1_sb[:], ct_sb[:], start=True, stop=True)
            # hT replicated at partition offsets 0/32/64/96 to match the
            # packed w2 k-groups; replication DMAs ride the gpsimd queue
            # (short) so they aren't stuck behind the bulk loads
            ht_sb = consts.tile([128, BPC], bf16)
            nc.scalar.activation(
                out=ht_sb[0:K2, :], in_=ph[:], func=AF.Relu, bias=b1_sb[:]
            )
            for g in range(1, 4):
                nc.gpsimd.dma_start(
                    out=ht_sb[32 * g : 32 * g + K2, :], in_=ht_sb[0:K2, :]
                )

            # ---- bulk x loads: 26-padded-row chunks (6.76KB descriptors),
            # top/bottom-half interleaved so conv group g's two image
            # halves arrive together, alternating the two HWDGE queues ----
            xe = HP * WP // 5

            def load_x(p, k, q):
                q.dma_start(
                    out=xps[p][:, k * xe : (k + 1) * xe],
                    in_=xs_d.ap()[2 * p : 2 * p + 2].rearrange(
                        "b c (k e) -> b c k e", e=xe
                    )[:, :, k, :],
                )

            for p in range(PAIRS):
                for i, k in enumerate([0, 2, 1, 3, 4]):
                    if p == 1 and k in (3, 4):
                        continue  # x1 tail rides gpsimd after the scatters
                    load_x(p, k, [nc.sync, nc.scalar][(5 * p + i) % 2])

            def _cast_dve(out, in_):
                nc.vector.tensor_copy(out, in_)

            def _cast_act(out, in_):
                nc.scalar.activation(out=out, in_=in_, func=AF.Copy)

            cast_engs = [_cast_dve, _cast_act]
            # Stage-A matmuls write psum at partition strip 32g via
            # tile_position=(32g, 32g): one [128, 512] psum tile carries 4
            # k-groups' blocks, so the psum->sbuf downcast is full-width.
            # adj layout: partition 32g+b, col (ab*8 + cl)*576 + t*64 + co,
            # holding w2 512-block i = t*8 + ab*4 + g (ci = 32ab+8g+cl).
            adj = consts.tile([128, NT * CIN * COUT // 4], bf16, name="adj")
            adj_c = adj.rearrange("p (q cl tc) -> p q cl tc", q=2, cl=8)
            for t in range(NT):
                for ab in range(2):
                    pa = ps.tile([128, 512], f32, tag="ps", bufs=8)
                    for g in range(4):
                        nc.tensor.matmul(
                            pa[32 * g : 32 * g + BPC, :],
                            ht_sb[32 * g : 32 * g + K2, :],
                            w2s[32 * g : 32 * g + K2,
                                (2 * t + ab) * 512 : (2 * t + ab + 1) * 512],
                            start=True,
                            stop=True,
                            tile_position=(32 * g, 32 * g),
                        )
                    cast_engs[(2 * t + ab) % 2](
                        adj_c[:, ab, :, t * 64 : (t + 1) * 64],
                        pa.rearrange("p (cl co) -> p cl co", cl=8),
                    )
            # scatter all taps at once: per (pair, sample, ab, g) one DMA,
            # [1, 4608] contiguous -> [8 partitions, 576] runs into
            # wnew[p][64s+ci, t*64+co]
            for p in range(PAIRS):
                wn_s = wnew[p].rearrange(
                    "(s q g cl) f -> s q g cl f", s=2, q=2, g=4
                )
                for s in range(2):
                    for ab in range(2):
                        for g in range(4):
                            nc.gpsimd.dma_start(
                                out=wn_s[s, ab, g],
                                in_=adj[32 * g + 2 * p + s : 32 * g + 2 * p + s + 1,
                                        ab * 4608 : (ab + 1) * 4608],
                            )
            load_x(1, 3, nc.gpsimd)
            load_x(1, 4, nc.gpsimd)

            # ---- stage B: per-pair conv in 64x64 PE tiled mode.
            # psum tile [128, 512] per sample: partitions 0:64 accumulate
            # out rows h0:h0+4, partitions 64:128 rows h0+64:h0+68; the
            # four (sample, half) tiles stream concurrently on the PE. ----
            store_qs = [nc.sync, nc.scalar, nc.gpsimd]
            for p in range(PAIRS):
                xp3 = xps[p].rearrange("p (h w) -> p h w", w=WP)
                for g in range(NGRP):
                    h0 = 4 * g
                    pss = [
                        ps.tile([128, 512], f32, tag="ps", bufs=8, name=f"po{p}_{g}_{s}")
                        for s in range(2)
                    ]
                    for t in range(NT):
                        kh, kw = divmod(t, 3)
                        r1 = h0 + kh
                        r2 = h0 + H2 + kh
                        for s in range(2):
                            wk = wnew[p][64 * s : 64 * s + 64, t * 64 : (t + 1) * 64]
                            nc.tensor.matmul(
                                pss[s][0:64, :],
                                wk,
                                xp3[64 * s : 64 * s + 64, r1 : r1 + 4, kw : kw + W],
                                start=(t == 0),
                                stop=(t == NT - 1),
                                tile_position=(64 * s, 0),
                            )
                            nc.tensor.matmul(
                                pss[s][64:128, :],
                                wk,
                                xp3[64 * s : 64 * s + 64, r2 : r2 + 4, kw : kw + W],
                                start=(t == 0),
                                stop=(t == NT - 1),
                                tile_position=(64 * s, 64),
                            )
                    for s in range(2):
                        os = opool.tile([128, 512], f32, name=f"os{p}_{g}_{s}", tag="os")
                        if (g + s) % 2 == 0:
                            nc.vector.tensor_scalar_add(os[:], pss[s][:], cb_sb[:])
                        else:
                            nc.scalar.activation(
                                out=os[:], in_=pss[s][:], func=AF.Identity,
                                bias=cb_sb[:],
                            )
                        b = 2 * p + s
                        q = store_qs[(2 * g + s) % 3]
                        q.dma_start(
                            out=out_d.ap()[b : b + 1, :, h0 : h0 + 4, :],
                            in_=os[0:64, :],
                        )
                        q.dma_start(
                            out=out_d.ap()[b : b + 1, :, h0 + H2 : h0 + H2 + 4, :],
                            in_=os[64:128, :],
                        )

    nc.compile()
    return nc


def _get_nc():
    if "nc" not in _CACHE:
        _CACHE["nc"] = _build()
    return _CACHE["nc"]


def _prep(x, c, conv_w, conv_b, mlp_w1, mlp_b1, mlp_w2, mlp_b2):
    x = np.ascontiguousarray(x, dtype=np.float32)
    c = np.ascontiguousarray(c, dtype=np.float32)
    conv_w = np.asarray(conv_w, dtype=np.float32)
    conv_b = np.asarray(conv_b, dtype=np.float32)
    mlp_w1 = np.asarray(mlp_w1, dtype=np.float32)
    mlp_b1 = np.asarray(mlp_b1, dtype=np.float32)
    mlp_w2 = np.asarray(mlp_w2, dtype=np.float32)
    mlp_b2 = np.asarray(mlp_b2, dtype=np.float32)

    # padded bf16 x, flattened spatial
    xsp = np.zeros((B, CIN, HP, WP), dtype=ml_dtypes.bfloat16)
    xsp[:, :, 1 : HP - 1, 1 : WP - 1] = x
    xsp = xsp.reshape(B, CIN, HP * WP)

    # w1' [CL1, K2]: [[w1, 0], [0, 1]]; cT' [CL1, BPC] gets a ones row
    w19 = np.zeros((CL1, K2), dtype=np.float32)
    w19[:CL, :MH] = mlp_w1
    w19[CL, MH] = 1.0
    b117 = np.concatenate([mlp_b1, np.zeros(1, np.float32)]).reshape(K2, 1)
    b117 = np.ascontiguousarray(b117, dtype=np.float32)

    # w2p[k, t, ci, co] = mlp_w2[k, co*576 + ci*9 + t]
    # row 16 = (mlp_b2 + conv_w), same permutation -> adj == full weight
    w2p = mlp_w2.reshape(MH, COUT, CIN, NT).transpose(0, 3, 2, 1)
    b2p = mlp_b2.reshape(COUT, CIN, NT).transpose(2, 1, 0)
    cwp = conv_w.reshape(COUT, CIN, NT).transpose(2, 1, 0)  # [t, ci, co]
    row16 = (b2p + cwp).reshape(1, -1)
    w2p = np.concatenate([w2p.reshape(MH, -1), row16], axis=0)
    # 512-col block i -> k-group i%4 at cols (i//4)*512 (round-robin so
    # each tap's 8 blocks hit all 4 PE row-groups); strips padded to all
    # 128 partitions (32g+k) so the load is full-width
    w2b = w2p.reshape(K2, 72, 512)
    w2g = np.zeros((128, W2COL), dtype=np.float32)
    for i in range(72):
        g = i % 4
        w2g[32 * g : 32 * g + K2, (i // 4) * 512 : (i // 4 + 1) * 512] = w2b[:, i, :]
    w2g = np.ascontiguousarray(w2g.astype(ml_dtypes.bfloat16))

    cb2 = np.ascontiguousarray(
        np.tile(conv_b.reshape(COUT, 1), (2, 1)), dtype=np.float32
    )

    in_maps = []
    for i in range(NCORES):
        sl = slice(i * BPC, (i + 1) * BPC)
        ct9 = np.concatenate([c[sl].T, np.ones((1, BPC), np.float32)], axis=0)
        in_maps.append(
            {
                "xsp": np.ascontiguousarray(xsp[sl]),
                "cT": np.ascontiguousarray(ct9),
                "w1": w19,
                "b1": b117,
                "w2p": w2g,
                "cb2": cb2,
            }
        )
    return in_maps


def _run(inputs, trace=False):
    from concourse.bass_utils import run_bass_kernel_spmd

    nc = _get_nc()
    in_maps = _prep(**inputs)
    res = run_bass_kernel_spmd(
        nc, in_maps, core_ids=list(range(NCORES)), trace=trace
    )
    out = np.concatenate([res.results[i]["out"] for i in range(NCORES)], axis=0)
    return out, res


def kernel(**inputs):
    out, _ = _run(inputs, trace=False)
    return out


# revision 29
# speedup vs baseline: 1.7250x; 1.0222x over previous
"""Trainium2 Bass kernel for conditional-adjustment conv (CAConv).

Per sample b: h = relu(c[b] @ mlp_w1 + mlp_b1); adj = h @ mlp_w2 + mlp_b2;
w[b] = conv_w + adj.reshape(Co,Ci,3,3); out[b] = conv2d(x[b], w[b], pad=1) + conv_b.

Sharding: data-parallel over batch, 4 samples per core on 8 cores (SPMD).

Per-core device kernel:
  Stage A (weight gen, bf16): one small matmul + relu produces hT[17,4]; the
  17th row is ones (host-appended ones rows in c/w1), so row 16 of w2p —
  which the host sets to mlp_b2 + conv_w, both permuted — rides along and
  the result is directly the complete per-sample conv weight. The w2
  columns are round-robined over the four 32-partition k-groups in 512-col
  blocks so each tap's 8 matmuls spread across all 4 PE row-groups and run
  concurrently. Each tap's psum block is DMA-scattered (fp32, no engine
  cast) into wtmp[p][b*64+ci, t*64+co]; one wide [128, 576] tensor_copy
  per pair then downcasts to the bf16 conv weights wnew[p].
  Stage B (conv): host-padded bf16 x (130x130) for a sample pair lives as
  [ci(2 samples), h, w] across 128 partitions. The 128x128 PE runs in
  64x64 tiled mode as FOUR concurrent tiles — (sbuf rows, psum cols) =
  (0,0)/(64,0)/(0,64)/(64,64) = (sample a, top half)/(b, top)/(a, bottom)/
  (b, bottom) — each a full-utilization [K=64ci, M=64co, N=512] matmul, so
  no block-diagonal zero padding. Each psum tile [128, 512] accumulates 9
  shift-taps for one sample: partitions 0:64 = out rows h0:h0+4,
  partitions 64:128 = out rows h0+64:h0+68. Bias is added during the
  PSUM->SBUF copy (alternating DVE / scalar engines), then two half-tile
  DMAs store to DRAM on the evac engine's own queue.

  DMA queues: sync = consts + w2 + bulk x loads (sync engine is otherwise
  idle, so pushes issue immediately; x chunks are 13 padded rows, ordered
  top/bottom-half interleaved so conv group g's two image halves arrive
  together); gpsimd/vector = per-tap weight scatters (split by pair);
  scalar+vector = output stores, each after its own engine's evac.
"""

import sys

if "/opt/trn_rl_repo" not in sys.path:
    sys.path.insert(0, "/opt/trn_rl_repo")

import numpy as np
import ml_dtypes

B = 32
NCORES = 8
BPC = B // NCORES          # samples per core = 4
PAIRS = BPC // 2           # sample pairs per core = 2
CIN = COUT = 64
H = W = 128
H2 = H // 2                # bottom-half row offset = 64
HP = WP = 130              # padded dims
KH = KW = 3
NT = KH * KW               # taps = 9
CL = 8                     # c length
CL1 = CL + 1               # + ones row
MH = 16                    # mlp hidden
K2 = MH + 1                # mlp hidden + ones row
NGRP = H2 // 4             # psum groups per pair = 16 (4 rows x 2 halves each)
W2COL = (NT * CIN * COUT) // 4   # w2 columns per k-group = 9216

_CACHE = {}


def _build():
    import concourse.bass as bass
    import concourse.mybir as mybir
    import concourse.tile as tile
    from concourse import bacc

    f32 = mybir.dt.float32
    bf16 = mybir.dt.bfloat16
    AF = mybir.ActivationFunctionType

    nc = bacc.Bacc("TRN2", target_bir_lowering=False, debug=False)

    xs_d = nc.dram_tensor("xsp", [BPC, CIN, HP * WP], bf16, kind="ExternalInput")
    ct_d = nc.dram_tensor("cT", [CL1, BPC], f32, kind="ExternalInput")
    w1_d = nc.dram_tensor("w1", [CL1, K2], f32, kind="ExternalInput")
    b1_d = nc.dram_tensor("b1", [K2, 1], f32, kind="ExternalInput")
    w2_d = nc.dram_tensor("w2p", [128, W2COL], bf16, kind="ExternalInput")
    cb_d = nc.dram_tensor("cb2", [128, 1], f32, kind="ExternalInput")
    out_d = nc.dram_tensor("out", [BPC, COUT, H, W], f32, kind="ExternalOutput")

    with tile.TileContext(nc) as tc:
        with (
            tc.tile_pool(name="consts", bufs=1) as consts,
            tc.tile_pool(name="xpool", bufs=2) as xpool,
            tc.tile_pool(name="opool", bufs=6) as opool,
            tc.tile_pool(name="pspool", bufs=1, space=bass.MemorySpace.PSUM) as ps,
        ):
            # ---- constants in (sync queue; kept small + early) ----
            ct_sb = consts.tile([CL1, BPC], f32)
            nc.sync.dma_start(out=ct_sb[:], in_=ct_d.ap())
            w1_sb = consts.tile([CL1, K2], f32)
            nc.sync.dma_start(out=w1_sb[:], in_=w1_d.ap())
            b1_sb = consts.tile([K2, 1], f32)
            nc.sync.dma_start(out=b1_sb[:], in_=b1_d.ap())
            cb_sb = consts.tile([128, 1], f32)
            nc.sync.dma_start(out=cb_sb[:], in_=cb_d.ap())

            # w2 host-padded to all 128 partitions (k-strips at 32g+k,
            # dead rows zero): full-width loads run ~10x faster than
            # 17-partition strip loads (per-partition write rate limit),
            # well worth the extra 1.1MB of dead-row traffic
            w2s = consts.tile([128, W2COL], bf16, name="w2s")
            w2e = W2COL // 4
            for k in range(4):
                [nc.sync, nc.scalar][k % 2].dma_start(
                    out=w2s[:, k * w2e : (k + 1) * w2e],
                    in_=w2_d.ap()[:, k * w2e : (k + 1) * w2e],
                )

            xps = []
            for p in range(PAIRS):
                xp = xpool.tile([128, HP * WP], bf16, name=f"xp{p}", tag="xp")
                xps.append(xp)

            # per-pair per-tap bf16 weight blocks [b*64+ci, t*64+co]
            wnew = []
            for p in range(PAIRS):
                wnew.append(consts.tile([128, NT * 64], bf16, name=f"wnew{p}"))

            # ---- stage A: conditioning MLP ----
            ph = ps.tile([K2, BPC], f32, tag="ps", bufs=8)
            nc.tensor.matmul(ph[:], w1_sb[:], ct_sb[:], start=True, stop=True)
            # hT replicated at partition offsets 0/32/64/96 to match the
            # packed w2 k-groups; replication DMAs ride the gpsimd queue
            # (short) so they aren't stuck behind the bulk loads
            ht_sb = consts.tile([128, BPC], bf16)
            nc.scalar.activation(
                out=ht_sb[0:K2, :], in_=ph[:], func=AF.Relu, bias=b1_sb[:]
            )
            for g in range(1, 4):
                nc.gpsimd.dma_start(
                    out=ht_sb[32 * g : 32 * g + K2, :], in_=ht_sb[0:K2, :]
                )

            # ---- bulk x loads: 26-padded-row chunks (6.76KB descriptors),
            # top/bottom-half interleaved so conv group g's two image
            # halves arrive together, alternating the two HWDGE queues ----
            xe = HP * WP // 5

            def load_x(p, k, q):
                q.dma_start(
                    out=xps[p][:, k * xe : (k + 1) * xe],
                    in_=xs_d.ap()[2 * p : 2 * p + 2].rearrange(
                        "b c (k e) -> b c k e", e=xe
                    )[:, :, k, :],
                )

            for p in range(PAIRS):
                for i, k in enumerate([0, 2, 1, 3, 4]):
                    if p == 1 and k in (3, 4):
                        continue  # x1 tail rides gpsimd after the scatters
                    load_x(p, k, [nc.sync, nc.scalar][(5 * p + i) % 2])

            def _cast_dve(out, in_):
                nc.vector.tensor_copy(out, in_)

            def _cast_act(out, in_):
                nc.scalar.activation(out=out, in_=in_, func=AF.Copy)

            cast_engs = [_cast_dve, _cast_act]
            # Stage-A matmuls write psum at partition strip 32g via
            # tile_position=(32g, 32g): one [128, 512] psum tile carries 4
            # k-groups' blocks, so the psum->sbuf downcast is full-width.
            # adj layout: partition 32g+b, col (ab*8 + cl)*576 + t*64 + co,
            # holding w2 512-block i = t*8 + 2g + ab (ci = 16g+8ab+cl), so
            # each g covers a contiguous 16-ci range and the scatter's
            # destination is a plain 16-partition slice.
            adj = consts.tile([128, NT * CIN * COUT // 4], bf16, name="adj")
            adj_c = adj.rearrange("p (q cl tc) -> p q cl tc", q=2, cl=8)
            for t in range(NT):
                for ab in range(2):
                    pa = ps.tile([128, 512], f32, tag="ps", bufs=8)
                    for g in range(4):
                        nc.tensor.matmul(
                            pa[32 * g : 32 * g + BPC, :],
                            ht_sb[32 * g : 32 * g + K2, :],
                            w2s[32 * g : 32 * g + K2,
                                (2 * t + ab) * 512 : (2 * t + ab + 1) * 512],
                            start=True,
                            stop=True,
                            tile_position=(32 * g, 32 * g),
                        )
                    cast_engs[(2 * t + ab) % 2](
                        adj_c[:, ab, :, t * 64 : (t + 1) * 64],
                        pa.rearrange("p (cl co) -> p cl co", cl=8),
                    )
            # scatter all taps at once: per (pair, g) one DMA with a
            # contiguous [2, 9216] source -> 32 partitions x 1152B runs
            # into wnew[p][64s+ci, t*64+co]; HWDGE queues (the SWDGE
            # per-DMA overhead would put ~12us between stage A and conv)
            for p in range(PAIRS):
                for s in range(2):
                    for g in range(4):
                        [nc.sync, nc.scalar][g % 2].dma_start(
                            out=wnew[p][64 * s + 16 * g : 64 * s + 16 * g + 16, :],
                            in_=adj[32 * g + 2 * p + s : 32 * g + 2 * p + s + 1, :],
                        )
            load_x(1, 3, nc.gpsimd)
            load_x(1, 4, nc.gpsimd)

            # ---- stage B: per-pair conv in 64x64 PE tiled mode.
            # psum tile [128, 512] per sample: partitions 0:64 accumulate
            # out rows h0:h0+4, partitions 64:128 rows h0+64:h0+68; the
            # four (sample, half) tiles stream concurrently on the PE. ----
            store_qs = [nc.sync, nc.scalar, nc.gpsimd]
            for p in range(PAIRS):
                xp3 = xps[p].rearrange("p (h w) -> p h w", w=WP)
                for g in range(NGRP):
                    h0 = 4 * g
                    pss = [
                        ps.tile([128, 512], f32, tag="ps", bufs=8, name=f"po{p}_{g}_{s}")
                        for s in range(2)
                    ]
                    for t in range(NT):
                        kh, kw = divmod(t, 3)
                        r1 = h0 + kh
                        r2 = h0 + H2 + kh
                        for s in range(2):
                            wk = wnew[p][64 * s : 64 * s + 64, t * 64 : (t + 1) * 64]
                            nc.tensor.matmul(
                                pss[s][0:64, :],
                                wk,
                                xp3[64 * s : 64 * s + 64, r1 : r1 + 4, kw : kw + W],
                                start=(t == 0),
                                stop=(t == NT - 1),
                                tile_position=(64 * s, 0),
                            )
                            nc.tensor.matmul(
                                pss[s][64:128, :],
                                wk,
                                xp3[64 * s : 64 * s + 64, r2 : r2 + 4, kw : kw + W],
                                start=(t == 0),
                                stop=(t == NT - 1),
                                tile_position=(64 * s, 64),
                            )
                    for s in range(2):
                        os = opool.tile([128, 512], f32, name=f"os{p}_{g}_{s}", tag="os")
                        if (g + s) % 2 == 0:
                            nc.vector.tensor_scalar_add(os[:], pss[s][:], cb_sb[:])
                        else:
                            nc.scalar.activation(
                                out=os[:], in_=pss[s][:], func=AF.Identity,
                                bias=cb_sb[:],
                            )
                        b = 2 * p + s
                        q = store_qs[(2 * g + s) % 3]
                        q.dma_start(
                            out=out_d.ap()[b : b + 1, :, h0 : h0 + 4, :],
                            in_=os[0:64, :],
                        )
                        q.dma_start(
                            out=out_d.ap()[b : b + 1, :, h0 + H2 : h0 + H2 + 4, :],
                            in_=os[64:128, :],
                        )

    nc.compile()
    return nc


def _get_nc():
    if "nc" not in _CACHE:
        _CACHE["nc"] = _build()
    return _CACHE["nc"]


def _prep(x, c, conv_w, conv_b, mlp_w1, mlp_b1, mlp_w2, mlp_b2):
    x = np.ascontiguousarray(x, dtype=np.float32)
    c = np.ascontiguousarray(c, dtype=np.float32)
    conv_w = np.asarray(conv_w, dtype=np.float32)
    conv_b = np.asarray(conv_b, dtype=np.float32)
    mlp_w1 = np.asarray(mlp_w1, dtype=np.float32)
    mlp_b1 = np.asarray(mlp_b1, dtype=np.float32)
    mlp_w2 = np.asarray(mlp_w2, dtype=np.float32)
    mlp_b2 = np.asarray(mlp_b2, dtype=np.float32)

    # padded bf16 x, flattened spatial
    xsp = np.zeros((B, CIN, HP, WP), dtype=ml_dtypes.bfloat16)
    xsp[:, :, 1 : HP - 1, 1 : WP - 1] = x
    xsp = xsp.reshape(B, CIN, HP * WP)

    # w1' [CL1, K2]: [[w1, 0], [0, 1]]; cT' [CL1, BPC] gets a ones row
    w19 = np.zeros((CL1, K2), dtype=np.float32)
    w19[:CL, :MH] = mlp_w1
    w19[CL, MH] = 1.0
    b117 = np.concatenate([mlp_b1, np.zeros(1, np.float32)]).reshape(K2, 1)
    b117 = np.ascontiguousarray(b117, dtype=np.float32)

    # w2p[k, t, ci, co] = mlp_w2[k, co*576 + ci*9 + t]
    # row 16 = (mlp_b2 + conv_w), same permutation -> adj == full weight
    w2p = mlp_w2.reshape(MH, COUT, CIN, NT).transpose(0, 3, 2, 1)
    b2p = mlp_b2.reshape(COUT, CIN, NT).transpose(2, 1, 0)
    cwp = conv_w.reshape(COUT, CIN, NT).transpose(2, 1, 0)  # [t, ci, co]
    row16 = (b2p + cwp).reshape(1, -1)
    w2p = np.concatenate([w2p.reshape(MH, -1), row16], axis=0)
    # 512-col block i -> k-group i%4 at cols (i//4)*512 (round-robin so
    # each tap's 8 blocks hit all 4 PE row-groups); strips padded to all
    # 128 partitions (32g+k) so the load is full-width
    w2b = w2p.reshape(K2, 72, 512)
    w2g = np.zeros((128, W2COL), dtype=np.float32)
    for i in range(72):
        t, i8 = divmod(i, 8)
        g, ab = divmod(i8, 2)
        w2g[32 * g : 32 * g + K2, (2 * t + ab) * 512 : (2 * t + ab + 1) * 512] = w2b[:, i, :]
    w2g = np.ascontiguousarray(w2g.astype(ml_dtypes.bfloat16))

    cb2 = np.ascontiguousarray(
        np.tile(conv_b.reshape(COUT, 1), (2, 1)), dtype=np.float32
    )

    in_maps = []
    for i in range(NCORES):
        sl = slice(i * BPC, (i + 1) * BPC)
        ct9 = np.concatenate([c[sl].T, np.ones((1, BPC), np.float32)], axis=0)
        in_maps.append(
            {
                "xsp": np.ascontiguousarray(xsp[sl]),
                "cT": np.ascontiguousarray(ct9),
                "w1": w19,
                "b1": b117,
                "w2p": w2g,
                "cb2": cb2,
            }
        )
    return in_maps


def _run(inputs, trace=False):
    from concourse.bass_utils import run_bass_kernel_spmd

    nc = _get_nc()
    in_maps = _prep(**inputs)
    res = run_bass_kernel_spmd(
        nc, in_maps, core_ids=list(range(NCORES)), trace=trace
    )
    out = np.concatenate([res.results[i]["out"] for i in range(NCORES)], axis=0)
    return out, res


def kernel(**inputs):
    out, _ = _run(inputs, trace=False)
    return out
